# revision 2
# baseline (speedup 1.0000x reference)
"""CascadeGDCN (3-hop graph diffusion conv) on 8 Trainium2 NeuronCores, v2.

Differences vs the v1 baseline (12.5ms):
  - bf16 everywhere in the SpMM path: X stored as [nodes, 128] bf16 rows
    (64 feats + 64 pad = 256B, the dma_gather minimum element), S one-hots
    and matmuls in bf16 (1cy/row on PE vs 4 for f32r), messages val-scaled
    in-place on the 64 real columns only.
  - Per-cell slot capacity = max over cores of ceil(count/128) (program
    must be uniform across cores) instead of a global worst-case cap:
    ~6% fewer descriptors / S elements / matmuls.
  - Gather calls = (block of 7 dest groups) x (4 source chunks), the four
    chunk calls issued on the 4 SWDGE queues concurrently: the Q7
    descriptor-generation pairs (2 cores per queue) run in parallel,
    which v1 left mostly idle (avg concurrency 1.6).
  - AllGathers in bf16-padded rows, dir0's collective overlaps dir1's
    compute and dir1's overlaps the next hop's dir0 compute.
  - denc/val tables SBUF-resident across hops; idx streams double
    buffered per call.
"""

import numpy as np

D = 64
NCORES = 8
NUM_HOPS = 3
N_NODES = 100000
SHARD = 12544                # dest rows per core (98 groups of 128)
NODES_PAD = SHARD * NCORES   # 100352
NCHUNKS = 4
CHUNK = NODES_PAD // NCHUNKS  # 25088 (< 32768 so chunk-local idx fits int16)
GROUPS = SHARD // 128        # 98
GPB = 7                      # dest groups per block
NBLOCKS = GROUPS // GPB      # 14


def _softmax(x):
    e = np.exp(x - x.max())
    return e / e.sum()


def _prep_direction(dest, src, val):
    """Per-core gather/S tables for one SpMM direction.

    Layout (identical across cores): [block][chunk][group-in-block][cap*128]
    with cap per cell = max over cores of ceil(count/128).

    Returns (tables_per_core, caps, call_off, tot):
      tables: idx int16 [128, tot/16], denc/val f32 [128, tot/128]
      caps: [block][chunk][gl] slot counts
      call_off: [block][chunk] edge-stream offset of each gather call
    """
    core = dest // SHARD
    d_loc_all = dest - core * SHARD
    g_all = d_loc_all >> 7
    c_all = src // CHUNK
    gl_all = g_all % GPB
    b_all = g_all // GPB
    ncells = NBLOCKS * NCHUNKS * GPB
    cell_all = (b_all * NCHUNKS + c_all) * GPB + gl_all

    # per-cell caps = max over cores
    counts = np.zeros((NCORES, ncells), dtype=np.int64)
    for m in range(NCORES):
        counts[m] = np.bincount(cell_all[core == m], minlength=ncells)
    caps_e = ((counts.max(axis=0) + 127) >> 7) << 7
    cell_base = np.zeros(ncells, dtype=np.int64)
    cell_base[1:] = np.cumsum(caps_e)[:-1]
    tot = int(caps_e.sum())

    caps = [[[int(caps_e[(b * NCHUNKS + c) * GPB + gl]) >> 7
              for gl in range(GPB)] for c in range(NCHUNKS)]
            for b in range(NBLOCKS)]
    call_off = [[int(cell_base[(b * NCHUNKS + c) * GPB])
                 for c in range(NCHUNKS)] for b in range(NBLOCKS)]

    out = []
    for m in range(NCORES):
        sel = core == m
        s = src[sel]
        v = val[sel].astype(np.float32)
        d_loc = d_loc_all[sel]
        cell = cell_all[sel]
        order = np.argsort(cell, kind="stable")
        cell_s = cell[order]
        starts = np.zeros(ncells, dtype=np.int64)
        starts[1:] = np.cumsum(counts[m])[:-1]
        rank = np.arange(cell_s.size) - starts[cell_s]
        pos = cell_base[cell_s] + rank

        idx_st = np.zeros(tot, dtype=np.int16)
        denc_st = np.full(tot, -1.0, dtype=np.float32)
        val_st = np.zeros(tot, dtype=np.float32)
        idx_st[pos] = (s[order] - c_all[sel][order] * CHUNK).astype(np.int16)
        denc_st[pos] = (d_loc[order] & 127).astype(np.float32)
        val_st[pos] = v[order]

        idx_tbl = np.tile(np.ascontiguousarray(idx_st.reshape(-1, 16).T),
                          (8, 1))
        denc_tbl = np.ascontiguousarray(denc_st.reshape(-1, 128).T)
        val_tbl = np.ascontiguousarray(val_st.reshape(-1, 128).T)
        out.append({"idx": idx_tbl, "denc": denc_tbl, "val": val_tbl})
    return out, caps, call_off, tot


def prep_host(H_l, edge_row, edge_col, edge_val, out_degree, in_degree,
              hop_attention, theta_out, theta_in, Theta):
    import ml_dtypes
    bf16 = ml_dtypes.bfloat16

    H = np.asarray(H_l, dtype=np.float32)
    er = np.asarray(edge_row, dtype=np.int64)
    ec = np.asarray(edge_col, dtype=np.int64)
    ev = np.asarray(edge_val, dtype=np.float32)
    od = np.asarray(out_degree, dtype=np.float32)
    idg = np.asarray(in_degree, dtype=np.float32)

    alpha = _softmax(np.asarray(hop_attention, dtype=np.float64))
    th_o = np.asarray(theta_out, dtype=np.float64)
    th_i = np.asarray(theta_in, dtype=np.float64)
    coef = [(float(alpha[k] * th_o[k]), float(alpha[k] * th_i[k]))
            for k in range(len(alpha))]

    # dir 0 ("out" chain): dest=row, src=col; dir 1: transposed
    t0, caps0, off0, tot0 = _prep_direction(er, ec, ev)
    t1, caps1, off1, tot1 = _prep_direction(ec, er, ev)

    x0o = np.zeros((NODES_PAD, 2 * D), dtype=bf16)
    x0i = np.zeros((NODES_PAD, 2 * D), dtype=bf16)
    x0o[:N_NODES, :D] = (np.maximum(od, 1e-8)[:, None] * H).astype(bf16)
    x0i[:N_NODES, :D] = (np.maximum(idg, 1e-8)[:, None] * H).astype(bf16)

    hpad = np.zeros((NODES_PAD, D), dtype=np.float32)
    hpad[:N_NODES] = H
    ident = np.eye(128, dtype=np.float32)
    theta = np.ascontiguousarray(
        np.asarray(Theta, dtype=np.float32).astype(bf16))

    in_maps = []
    for m in range(NCORES):
        in_maps.append({
            "x0_out": x0o,
            "x0_in": x0i,
            "hfm": np.ascontiguousarray(hpad[m * SHARD:(m + 1) * SHARD].T),
            "theta": theta,
            "ident": ident,
            "idx0": t0[m]["idx"],
            "denc0": t0[m]["denc"].astype(bf16),
            "val0": t0[m]["val"].astype(bf16),
            "idx1": t1[m]["idx"],
            "denc1": t1[m]["denc"].astype(bf16),
            "val1": t1[m]["val"].astype(bf16),
        })
    meta = {"coef": coef, "caps": [caps0, caps1],
            "call_off": [off0, off1], "tot": [tot0, tot1]}
    return in_maps, meta


def build_program(tc, ins, outs, meta):
    import concourse.mybir as mybir

    nc = tc.nc
    f32 = mybir.dt.float32
    bf16 = mybir.dt.bfloat16
    i16 = mybir.dt.int16
    EQ, MUL, ADD = (mybir.AluOpType.is_equal, mybir.AluOpType.mult,
                    mybir.AluOpType.add)

    coef = meta["coef"]
    caps = meta["caps"]            # [dir][block][chunk][gl]
    call_off = meta["call_off"]    # [dir][block][chunk]
    tot = meta["tot"]              # [dir]
    max_ns_call = max(
        sum(caps[d][b][c][gl] for gl in range(GPB))
        for d in range(2) for b in range(NBLOCKS) for c in range(NCHUNKS))

    rg = [list(range(NCORES))]

    bounce = [nc.dram_tensor(f"bounce{d}", [SHARD, 2 * D], bf16,
                             kind="Internal") for d in range(2)]
    xbuf = [[nc.dram_tensor(f"xbuf{d}_{p}", [NODES_PAD, 2 * D], bf16,
                            kind="Internal", addr_space="Shared")
             for p in range(2)] for d in range(2)]

    tabs = [
        (ins["idx0"], ins["denc0"], ins["val0"]),
        (ins["idx1"], ins["denc1"], ins["val1"]),
    ]
    x0 = [ins["x0_out"], ins["x0_in"]]

    with (
        tc.tile_pool(name="const", bufs=1) as cpool,
        tc.tile_pool(name="tabs", bufs=2) as tpool,
        tc.tile_pool(name="idxs", bufs=2) as ipool,
        tc.tile_pool(name="msgs", bufs=2) as mpool,
        tc.tile_pool(name="sbld", bufs=2) as spool,
        tc.tile_pool(name="xn", bufs=1) as xpool,
        tc.tile_pool(name="fin", bufs=2) as fpool,
        tc.tile_pool(name="ps", bufs=4, space="PSUM") as pspool,
        tc.tile_pool(name="psf", bufs=2, space="PSUM") as psfpool,
    ):
        iota_f = cpool.tile([128, 128], f32, tag="iota_f")
        nc.gpsimd.iota(iota_f[:], pattern=[[1, 128]], base=0,
                       channel_multiplier=0,
                       allow_small_or_imprecise_dtypes=True)
        iota = cpool.tile([128, 128], bf16, tag="iota")
        nc.scalar.copy(out=iota[:], in_=iota_f[:])
        ident_s = cpool.tile([128, 128], f32, tag="ident")
        nc.sync.dma_start(ident_s[:], ins["ident"][:])
        theta_s = cpool.tile([64, D], bf16, tag="theta")
        nc.sync.dma_start(theta_s[:], ins["theta"][:])

        st = cpool.tile([128, GROUPS, D], f32, tag="st")
        nc.vector.memset(st[:], 0.0)

        for hop in range(NUM_HOPS):
            for dirn in range(2):
                idx_d, denc_d, val_d = tabs[dirn]
                xsrc = (x0[dirn] if hop == 0
                        else xbuf[dirn][(hop - 1) % 2].ap())

                xn = xpool.tile([128, GROUPS, D], bf16, tag="xn")

                for b in range(NBLOCKS):
                    per_call = []
                    for c in range(NCHUNKS):
                        eoff = call_off[dirn][b][c]
                        ns = sum(caps[dirn][b][c][gl] for gl in range(GPB))
                        L = ns * 128
                        if ns == 0:
                            per_call.append((None, None))
                            continue
                        idx_t = ipool.tile([128, max_ns_call * 8], i16,
                                           tag=f"idx{c}")  # L/16 cols used
                        nc.sync.dma_start(
                            idx_t[:, :L // 16],
                            idx_d[:, eoff // 16:(eoff + L) // 16])
                        msgs = mpool.tile([128, max_ns_call, 2 * D], bf16,
                                          tag=f"msgs{c}")
                        nc.gpsimd.dma_gather(
                            out_ap=msgs[:, :ns, :],
                            in_ap=xsrc[c * CHUNK:(c + 1) * CHUNK, :],
                            idxs_ap=idx_t[:, :L // 16],
                            num_idxs=L,
                            num_idxs_reg=L,
                            elem_size=2 * D,
                            single_packet=False,
                            queue_num=c,
                        )
                        soff = eoff // 128
                        dvt = tpool.tile([128, 2 * max_ns_call], bf16,
                                         tag=f"dv{c}")
                        nc.sync.dma_start(
                            dvt[:, :ns], denc_d[:, soff:soff + ns])
                        nc.sync.dma_start(
                            dvt[:, max_ns_call:max_ns_call + ns],
                            val_d[:, soff:soff + ns])
                        S = spool.tile([128, max_ns_call, 128], bf16,
                                       tag=f"S{c}")
                        iota_v = iota[:].rearrange(
                            "p (o c) -> p o c", o=1).broadcast_to(
                            [128, ns, 128])
                        nc.vector.tensor_tensor(
                            out=S[:, :ns, :], in0=iota_v,
                            in1=dvt[:, :ns].broadcast_to(
                                [128, ns, 128]),
                            op=EQ)
                        # fold edge_val into the 64 real message columns
                        nc.vector.tensor_tensor(
                            out=msgs[:, :ns, :D], in0=msgs[:, :ns, :D],
                            in1=dvt[:, max_ns_call:max_ns_call + ns]
                            .broadcast_to([128, ns, D]),
                            op=MUL)
                        per_call.append((msgs, S))

                    for gl in range(GPB):
                        g = b * GPB + gl
                        slots = []
                        for c in range(NCHUNKS):
                            msgs, S = per_call[c]
                            s0 = sum(caps[dirn][b][c][x] for x in range(gl))
                            for s in range(caps[dirn][b][c][gl]):
                                slots.append((msgs, S, s0 + s))
                        ps = pspool.tile([128, D], f32, tag="ps")
                        for i, (msgs, S, sl) in enumerate(slots):
                            nc.tensor.matmul(
                                ps[:],
                                lhsT=S[:, sl, :],
                                rhs=msgs[:, sl, :D],
                                start=(i == 0),
                                stop=(i == len(slots) - 1),
                            )
                        nc.scalar.copy(out=xn[:, g, :], in_=ps[:])
                        nc.vector.scalar_tensor_tensor(
                            out=st[:, g, :], in0=ps[:],
                            scalar=coef[hop][dirn], in1=st[:, g, :],
                            op0=MUL, op1=ADD)

                # xnew -> local bounce (node-major bf16, pad cols untouched)
                bounce_v = bounce[dirn].ap().rearrange(
                    "(g p) f -> p g f", p=128)[:, :, :D]
                nc.sync.dma_start(bounce_v, xn[:])
                if hop < NUM_HOPS - 1:
                    nc.gpsimd.collective_compute(
                        "AllGather", mybir.AluOpType.bypass,
                        replica_groups=rg,
                        ins=[bounce[dirn].ap().opt()],
                        outs=[xbuf[dirn][hop % 2].ap().opt()],
                    )

        # final: y_fm = sigmoid(Theta^T @ st_fm) + H_fm, feature-major
        fchunks = [(i * 4, min(4, GROUPS - i * 4))
                   for i in range((GROUPS + 3) // 4)]
        for gs, gcnt in fchunks:
            width = gcnt * 128
            stfm = fpool.tile([64, 4 * 128], bf16, tag="stfm")
            for j in range(gcnt):
                pt = psfpool.tile([64, 128], f32, tag="pt")
                nc.tensor.transpose(pt[:], st[:, gs + j, :], ident_s[:])
                nc.scalar.copy(out=stfm[:, j * 128:(j + 1) * 128], in_=pt[:])
            zp = psfpool.tile([64, 4 * 128], f32, tag="zp")
            nc.tensor.matmul(zp[:, :width], lhsT=theta_s[:],
                             rhs=stfm[:, :width], start=True, stop=True)
            sg = fpool.tile([64, 4 * 128], f32, tag="sg")
            nc.scalar.activation(sg[:, :width], zp[:, :width],
                                 mybir.ActivationFunctionType.Sigmoid)
            hf = fpool.tile([64, 4 * 128], f32, tag="hf")
            nc.sync.dma_start(
                hf[:, :width], ins["hfm"][:, gs * 128:gs * 128 + width])
            yt = fpool.tile([64, 4 * 128], f32, tag="yt")
            nc.vector.tensor_tensor(out=yt[:, :width], in0=sg[:, :width],
                                    in1=hf[:, :width], op=ADD)
            nc.sync.dma_start(
                outs["y"][:, gs * 128:gs * 128 + width], yt[:, :width])


def kernel(**inputs) -> np.ndarray:
    return _run(inputs, trace=False)[0]


def kernel_traced(inputs, trace_kwargs=None):
    return _run(inputs, trace=True, trace_kwargs=trace_kwargs or {})


def _run(inputs, trace=False, trace_kwargs=None):
    import concourse.bacc as bacc
    import concourse.mybir as mybir
    import concourse.tile as tile
    from concourse.bass_utils import run_bass_kernel_spmd

    in_maps, meta = prep_host(**inputs)

    nc = bacc.Bacc("TRN2", target_bir_lowering=False, debug=False,
                   num_devices=NCORES, num_swdge_queues=4)
    f32 = mybir.dt.float32
    bf16 = mybir.dt.bfloat16
    i16 = mybir.dt.int16
    tot = meta["tot"]

    ins = {}
    shapes = {
        "x0_out": ([NODES_PAD, 2 * D], bf16),
        "x0_in": ([NODES_PAD, 2 * D], bf16),
        "hfm": ([D, SHARD], f32),
        "theta": ([D, D], bf16),
        "ident": ([128, 128], f32),
        "idx0": ([128, tot[0] // 16], i16),
        "denc0": ([128, tot[0] // 128], bf16),
        "val0": ([128, tot[0] // 128], bf16),
        "idx1": ([128, tot[1] // 16], i16),
        "denc1": ([128, tot[1] // 128], bf16),
        "val1": ([128, tot[1] // 128], bf16),
    }
    for k, (shape, dt) in shapes.items():
        ins[k] = nc.dram_tensor(k, shape, dt, kind="ExternalInput").ap()
    y = nc.dram_tensor("y", [D, SHARD], f32, kind="ExternalOutput")

    with tile.TileContext(nc) as tc:
        build_program(tc, ins, {"y": y.ap()}, meta)
    nc.compile()

    kw = {}
    if trace:
        kw = dict(trace=True, trace_kwargs=trace_kwargs or {})
    res = run_bass_kernel_spmd(nc, in_maps, core_ids=list(range(NCORES)),
                               **kw)
    shards = [r["y"].T for r in res.results]  # each [SHARD, 64]
    out = np.concatenate(shards, axis=0)[:N_NODES]
    return np.ascontiguousarray(out.astype(np.float32)), res
